# revision 34
# baseline (speedup 1.0000x reference)
"""Trainium2 Bass kernel for ContextAwareArtRecSys (gnn_message_passing).

Math fold: the reference is
    score[e] = concat(z_u[src] @ Wu.T + bu, z_i[dst] @ Wi.T + bi) @ wo.T + bo
Everything after the gather is linear, so with
    vu = wo[:, :128] @ Wu,  vi = wo[:, 128:] @ Wi          (256-vectors)
    c  = wo[:, :128]@bu + wo[:, 128:]@bi + bo              (scalar)
we have score[e] = (z_u @ vu)[src] + (z_i @ vi)[dst] + c.

v3 design — no per-edge indirect-DMA descriptors (SWDGE service is
hard-serialized at ~4.2ns/descriptor, measured):

  1. Node scores z@v on the PE: z is shipped transposed in 512-node
     chunks, v is a 1-column stationary, each chunk accumulates h-halves
     into a [1,512] PSUM row which the Activation engine drains to a
     bf16 score row in SBUF.  One combined AllGather publishes every
     core's (users | items) score block.
  2. Each core covers 62,500 edges in TWO row-bucketed layouts:
     space A rows = user-table position // 392 (user side), space B
     rows = item-table position // 784.  Each row's node scores are a
     STATIC contiguous slice of the AllGathered table -> plain DMAs.
  3. Per-row expansion: local_scatter (GPSIMD ucode, per-partition int16
     indices, ~4.3ns/idx) drops each node score at its first-edge slot,
     then a masked DVE scan (state = M*state + V) fills each segment.
  4. The item side is routed from space B to space A with
     local_scatter -> 16 PE tile-transposes -> local_scatter
     (any (q,j)->(p,f) permutation = row-permute, transpose, row-permute).
  5. One fused DVE op adds user + item + c; result DMAs out.
"""

import numpy as np

N_CORES = 8
N_USERS, N_ITEMS, E, H = 50000, 100000, 500000, 256
HALF = H // 2
EC = E // N_CORES                  # 62500 edges per core

U_OWN = 6272                       # users owned per core (8*6272 = 50176)
I_OWN = 12544                      # items owned per core (8*12544 = 100352)
S_OWN = U_OWN + I_OWN              # 18816 per-core score block
UCH = 13                           # user psum chunks of 512 (6656 slots)
ICH = 25                           # item psum chunks of 512 (12800 slots)
TCH = UCH + ICH                    # 38
USLOT = ICH * 512                  # 12800: user slots start (items first)
UA = 392                           # user-table entries per A-row (16/core)
IA = 784                           # item-table entries per B-row (16/core)
F = 640                            # edge slots per row
STG = 2046                         # staging cols (local_scatter dst < 2048)
STG2 = 2048                        # post-transpose staging cols

_CACHE = {}


def _build():
    if "nc" in _CACHE:
        return _CACHE["nc"]
    import concourse.bass as bass
    import concourse.tile as tile
    import concourse.mybir as mybir
    from concourse import bacc

    f32 = mybir.dt.float32
    bf16 = mybir.dt.bfloat16
    i16 = mybir.dt.int16

    nc = bacc.Bacc("TRN2", target_bir_lowering=False, debug=False,
                   num_devices=N_CORES)

    zt = nc.dram_tensor("zt", [128, TCH * 1024], bf16, kind="ExternalInput")
    w_user = nc.dram_tensor("w_user", [HALF, H], f32, kind="ExternalInput")
    w_item = nc.dram_tensor("w_item", [HALF, H], f32, kind="ExternalInput")
    wo_u = nc.dram_tensor("wo_u", [HALF, 1], f32, kind="ExternalInput")
    wo_i = nc.dram_tensor("wo_i", [HALF, 1], f32, kind="ExternalInput")
    b_user = nc.dram_tensor("b_user", [HALF, 1], f32, kind="ExternalInput")
    b_item = nc.dram_tensor("b_item", [HALF, 1], f32, kind="ExternalInput")
    b_out = nc.dram_tensor("b_out", [1, 1], f32, kind="ExternalInput")
    identd = nc.dram_tensor("identd", [128, 128], bf16, kind="ExternalInput")
    lsa = nc.dram_tensor("lsa", [128, UA], i16, kind="ExternalInput")
    lsb = nc.dram_tensor("lsb", [128, IA], i16, kind="ExternalInput")
    p1i = nc.dram_tensor("p1i", [128, F], i16, kind="ExternalInput")
    p2i = nc.dram_tensor("p2i", [128, STG2], i16, kind="ExternalInput")
    ma = nc.dram_tensor("ma", [128, F], bf16, kind="ExternalInput")
    mb = nc.dram_tensor("mb", [128, F], bf16, kind="ExternalInput")
    out = nc.dram_tensor("out", [128, F], f32, kind="ExternalOutput")

    warm_loc = nc.dram_tensor("warm_loc", [N_CORES, 1], bf16)
    warm_all = nc.dram_tensor("warm_all", [N_CORES, 1], bf16)
    su_loc = nc.dram_tensor("su_loc", [N_CORES * U_OWN, 1], bf16)
    si_loc = nc.dram_tensor("si_loc", [N_CORES * I_OWN, 1], bf16)
    su_all = nc.dram_tensor("su_all", [N_CORES * U_OWN, 1], bf16)
    si_all = nc.dram_tensor("si_all", [N_CORES * I_OWN, 1], bf16)

    groups = [list(range(N_CORES))]

    with tile.TileContext(nc) as tc:
        with (
            tc.tile_pool(name="consts", bufs=1) as consts,
            tc.tile_pool(name="zpool", bufs=4) as zpool,
            tc.tile_pool(name="work", bufs=1) as work,
            tc.tile_pool(name="psum", bufs=1, space="PSUM") as psum,
            tc.tile_pool(name="spsum", bufs=4, space="PSUM") as spsum,
        ):
            # ---- warm up the collective engine (absorbs cross-core
            # sync + CC setup under phase 1) ----
            nc.gpsimd.collective_compute(
                "AllToAll", mybir.AluOpType.bypass,
                replica_groups=groups, ins=[warm_loc.ap()],
                outs=[warm_all.ap()],
            )

            # ---- weights for the fold ----
            wu_t = consts.tile([HALF, H], f32)
            nc.sync.dma_start(wu_t[:], w_user.ap())
            wi_t = consts.tile([HALF, H], f32)
            nc.sync.dma_start(wi_t[:], w_item.ap())
            wou_t = consts.tile([HALF, 1], f32)
            nc.sync.dma_start(wou_t[:], wo_u.ap())
            woi_t = consts.tile([HALF, 1], f32)
            nc.sync.dma_start(woi_t[:], wo_i.ap())
            bu_t = consts.tile([HALF, 1], f32)
            nc.sync.dma_start(bu_t[:], b_user.ap())
            bi_t = consts.tile([HALF, 1], f32)
            nc.sync.dma_start(bi_t[:], b_item.ap())
            bo_t = consts.tile([1, 1], f32)
            nc.sync.dma_start(bo_t[:], b_out.ap())


            # ---- fold: vT columns (vu0 | vu1 | vi0 | vi1), and c ----
            vps = psum.tile([128, 4], f32, tag="vps")
            nc.tensor.matmul(vps[:, 0:1], wu_t[:, 0:HALF], wou_t[:],
                             start=True, stop=True)
            nc.tensor.matmul(vps[:, 1:2], wu_t[:, HALF:H], wou_t[:],
                             start=True, stop=True)
            nc.tensor.matmul(vps[:, 2:3], wi_t[:, 0:HALF], woi_t[:],
                             start=True, stop=True)
            nc.tensor.matmul(vps[:, 3:4], wi_t[:, HALF:H], woi_t[:],
                             start=True, stop=True)
            vT = consts.tile([128, 4], bf16)
            nc.vector.tensor_copy(vT[:], vps[:])

            ones_k1 = consts.tile([HALF, 128], f32)
            nc.vector.memset(ones_k1[:], 1.0)
            bub = consts.tile([HALF, 128], f32)
            nc.vector.tensor_scalar_mul(bub[:], ones_k1[:], bu_t[:])
            bib = consts.tile([HALF, 128], f32)
            nc.vector.tensor_scalar_mul(bib[:], ones_k1[:], bi_t[:])
            cps = psum.tile([128, 1], f32, tag="cps")
            nc.tensor.matmul(cps[:], bub[:], wou_t[:], start=True, stop=False)
            nc.tensor.matmul(cps[:], bib[:], woi_t[:], start=False, stop=False)
            nc.tensor.matmul(cps[:], ones_k1[0:1, :], bo_t[:],
                             start=False, stop=True)
            c_t = consts.tile([128, 1], f32)
            nc.vector.tensor_copy(c_t[:], cps[:])

            # ---- node scores on PE, drained by Activation to bf16 row ----
            # chunks grouped 4 per DMA, alternating between the two HWDGE
            # queues (SP / Activation) to split the z bandwidth.
            srow = work.tile([1, TCH * 512], bf16)
            GRP = 4
            usl = work.tile([128, UA], bf16)
            ua_t = work.tile([128, F], bf16)
            uexp = work.tile([128, F], bf16)
            for g in range(0, TCH, GRP):
                n = min(GRP, TCH - g)
                zch = zpool.tile([128, GRP * 1024], bf16, tag="z")
                zeng = nc.sync if (g // GRP) % 2 == 0 else nc.scalar
                zeng.dma_start(
                    zch[:, 0:n * 1024],
                    zt.ap()[:, g * 1024:(g + n) * 1024],
                )
                pss = [spsum.tile([1, 512], f32, tag="sc", name=f"ps_{g}_{j}")
                       for j in range(n)]
                for j in range(n):
                    a = 2 if g + j < ICH else 0
                    nc.tensor.matmul(pss[j][:], vT[:, a:a + 1],
                                     zch[:, j * 1024:j * 1024 + 512],
                                     start=True, stop=False)
                for j in range(n):
                    a = 2 if g + j < ICH else 0
                    nc.tensor.matmul(pss[j][:], vT[:, a + 1:a + 2],
                                     zch[:, j * 1024 + 512:(j + 1) * 1024],
                                     start=False, stop=True)
                for j in range(n):
                    c = g + j
                    if c % 2 == 0:
                        nc.vector.tensor_copy(
                            srow[0:1, c * 512:(c + 1) * 512], pss[j][:])
                    else:
                        nc.scalar.copy(
                            srow[0:1, c * 512:(c + 1) * 512], pss[j][:])

            # items done first: replicate + AllToAll as soon as drains land
            for rr in range(N_CORES):
                nc.sync.dma_start(
                    si_loc.ap()[rr * I_OWN:(rr + 1) * I_OWN]
                    .rearrange("(a b) one -> a (b one)", a=1),
                    srow[0:1, 0:I_OWN],
                )
            nc.gpsimd.collective_compute(
                "AllToAll", mybir.AluOpType.bypass,
                replica_groups=groups,
                ins=[si_loc.ap()],
                outs=[si_all.ap()],
            )

            # aux inputs for the expansion/routing phases (sync queue,
            # overlapping the collectives)
            ident = consts.tile([128, 128], bf16)
            nc.sync.dma_start(ident[:], identd.ap())
            lsa_t = consts.tile([128, UA], i16)
            nc.sync.dma_start(lsa_t[:], lsa.ap())
            lsb_t = consts.tile([128, IA], i16)
            nc.sync.dma_start(lsb_t[:], lsb.ap())
            p1_t = consts.tile([128, F], i16)
            nc.sync.dma_start(p1_t[:], p1i.ap())
            p2_t = consts.tile([128, STG2], i16)
            nc.sync.dma_start(p2_t[:], p2i.ap())
            ma_t = consts.tile([128, F], bf16)
            nc.sync.dma_start(ma_t[:], ma.ap())
            mb_t = consts.tile([128, F], bf16)
            nc.sync.dma_start(mb_t[:], mb.ap())
            for rr in range(N_CORES):
                nc.scalar.dma_start(
                    su_loc.ap()[rr * U_OWN:(rr + 1) * U_OWN]
                    .rearrange("(a b) one -> a (b one)", a=1),
                    srow[0:1, USLOT:USLOT + U_OWN],
                )
            cc_u = nc.gpsimd.collective_compute(
                "AllToAll", mybir.AluOpType.bypass,
                replica_groups=groups,
                ins=[su_loc.ap()],
                outs=[su_all.ap()],
            )
            isl = work.tile([128, IA], bf16)
            nc.sync.dma_start(
                isl[:],
                si_all.ap().rearrange("(p a) one -> p (a one)", p=128),
            )
            nc.scalar.dma_start(
                usl[:],
                su_all.ap().rearrange("(p a) one -> p (a one)", p=128),
            )

            # ---- item side: seg-start scatter + scan (space B) ----
            ib_t = work.tile([128, F], bf16)
            lsb_i = nc.gpsimd.local_scatter(ib_t[:], isl[:], lsb_t[:],
                                            128, F, IA)
            bass._add_dep_helper(
                lsb_i.ins, cc_u.ins, sync=True,
                reason="keep AG-U trigger ahead of the GPSIMD scatter chain")
            iexp = work.tile([128, F], bf16)
            nc.vector.tensor_tensor_scan(
                iexp[:], mb_t[:], ib_t[:], 0.0,
                mybir.AluOpType.mult, mybir.AluOpType.add,
            )

            # ---- route item scores B -> A: ls, transpose, ls ----
            stg = work.tile([128, STG], bf16)
            nc.gpsimd.local_scatter(stg[:], iexp[:], p1_t[:], 128, STG, F)
            # user expansion fills the GPSIMD gap while PE transposes run
            nc.gpsimd.local_scatter(ua_t[:], usl[:], lsa_t[:], 128, F, UA)
            nc.vector.tensor_tensor_scan(
                uexp[:], ma_t[:], ua_t[:], 0.0,
                mybir.AluOpType.mult, mybir.AluOpType.add,
            )
            pt = psum.tile([128, STG2], bf16, tag="pt")
            for t in range(15):
                nc.tensor.transpose(
                    pt[:, t * 128:(t + 1) * 128],
                    stg[:, t * 128:(t + 1) * 128], ident[:]
                )
            nc.tensor.transpose(
                pt[0:STG - 1920, 1920:2048], stg[:, 1920:STG], ident[:]
            )
            stg2 = work.tile([128, STG2], bf16)
            nc.vector.tensor_copy(stg2[:], pt[:])
            iex2 = work.tile([128, F], bf16)
            nc.gpsimd.local_scatter(iex2[:], stg2[:], p2_t[:], 128, F, STG2)

            # ---- combine: out = (uexp + c) + iex2 ----
            outf = work.tile([128, F], f32)
            nc.vector.scalar_tensor_tensor(
                outf[:], uexp[:], c_t[:], iex2[:],
                mybir.AluOpType.add, mybir.AluOpType.add,
            )
            nc.sync.dma_start(out.ap(), outf[:])

    nc.compile()
    _CACHE["nc"] = nc
    return nc


def _swizzle_zt(zu_sh, zi_sh):
    """[128, TCH*1024] bf16: chunk c = 512-node block; cols = h0|h1 halves.
    zT[h, 1024c + half*512 + j] = z[node(512c + j), half*128 + h]."""
    import ml_dtypes

    zu = np.zeros((UCH * 512, H), dtype=np.float32)
    zu[:zu_sh.shape[0]] = zu_sh
    zi = np.zeros((ICH * 512, H), dtype=np.float32)
    zi[:zi_sh.shape[0]] = zi_sh
    zall = np.concatenate([zi, zu], axis=0)            # items first
    zr = zall.reshape(TCH, 512, 2, 128).transpose(3, 0, 2, 1)
    return np.ascontiguousarray(
        zr.reshape(128, TCH * 1024)
    ).astype(ml_dtypes.bfloat16)


def _tpos_u(u):
    return u


def _tpos_i(d):
    return d


def _rank_in_group(keys):
    order = np.argsort(keys, kind="stable")
    ks = keys[order]
    first = np.r_[True, ks[1:] != ks[:-1]]
    gstart = np.where(first)[0]
    ranks_sorted = np.arange(len(keys)) - np.repeat(
        gstart, np.diff(np.r_[gstart, len(keys)])
    )
    ranks = np.empty(len(keys), dtype=np.int64)
    ranks[order] = ranks_sorted
    return ranks


def _pack_core2(tpu, tpi):
    """Host-side index construction for one core's edges, from the
    table positions (tpu/tpi) of each edge's endpoints."""
    # user side rows: tpos = S_OWN*k + local, local < U_OWN
    paE = tpu // UA
    oaE = tpu % UA                               # offset within row slice
    keyA = paE * (UA + 1) + oaE                  # sort key: (row, offset)
    orderA = np.argsort(keyA, kind="stable")
    paS = paE[orderA]
    oaS = oaE[orderA]
    rowstartA = np.searchsorted(paS, np.arange(128))
    jaS = np.arange(EC) - rowstartA[paS]
    assert jaS.max() < F, f"A row overflow {jaS.max()}"
    firstA = np.r_[True, (paS[1:] != paS[:-1]) | (oaS[1:] != oaS[:-1])]
    lsa_m = np.full((128, UA), -1, dtype=np.int16)
    lsa_m[paS[firstA], oaS[firstA]] = jaS[firstA]
    ma_m = np.ones((128, F), dtype=np.float32)
    ma_m[paS[firstA], jaS[firstA]] = 0.0

    posA = np.empty(EC, dtype=np.int64)
    posA[orderA] = np.arange(EC)
    jaE = posA - rowstartA[paE]

    # item side rows
    qbE = tpi // IA
    obE = tpi % IA
    keyB = qbE * (IA + 1) + obE
    orderB = np.argsort(keyB, kind="stable")
    qbS = qbE[orderB]
    obS = obE[orderB]
    rowstartB = np.searchsorted(qbS, np.arange(128))
    jbS = np.arange(EC) - rowstartB[qbS]
    assert jbS.max() < F, f"B row overflow {jbS.max()}"
    firstB = np.r_[True, (qbS[1:] != qbS[:-1]) | (obS[1:] != obS[:-1])]
    lsb_m = np.full((128, IA), -1, dtype=np.int16)
    lsb_m[qbS[firstB], obS[firstB]] = jbS[firstB]
    mb_m = np.ones((128, F), dtype=np.float32)
    mb_m[qbS[firstB], jbS[firstB]] = 0.0

    posB = np.empty(EC, dtype=np.int64)
    posB[orderB] = np.arange(EC)
    jbE = posB - rowstartB[qbE]

    # routing: staging col = 128*t + paE on row qbE; after transpose the
    # value sits at (paE, 128*t + qbE) and moves to final slot jaE.
    tE = _rank_in_group(qbE * 128 + paE)
    scol = 128 * tE + paE
    assert scol.max() < STG, f"staging overflow {scol.max()}"
    p1_m = np.full((128, F), -1, dtype=np.int16)
    p1_m[qbE, jbE] = scol
    p2_m = np.full((128, STG2), -1, dtype=np.int16)
    p2_m[paE, 128 * tE + qbE] = jaE

    return {
        "lsa": lsa_m, "lsb": lsb_m, "p1i": p1_m, "p2i": p2_m,
        "ma": ma_m, "mb": mb_m, "paE": paE, "jaE": jaE,
    }


def _make_in_maps(inputs):
    import ml_dtypes

    z_user = np.asarray(inputs["z_user"], dtype=np.float32)
    z_item = np.asarray(inputs["z_item"], dtype=np.float32)
    src = np.asarray(inputs["edge_src"]).astype(np.int64)
    dst = np.asarray(inputs["edge_dst"]).astype(np.int64)
    w_user = np.asarray(inputs["w_user"], dtype=np.float32)
    w_item = np.asarray(inputs["w_item"], dtype=np.float32)
    b_user = np.asarray(inputs["b_user"], dtype=np.float32).reshape(HALF, 1)
    b_item = np.asarray(inputs["b_item"], dtype=np.float32).reshape(HALF, 1)
    w_out = np.asarray(inputs["w_out"], dtype=np.float32)
    b_out = np.asarray(inputs["b_out"], dtype=np.float32).reshape(1, 1)
    wo_u = w_out[0, :HALF].reshape(HALF, 1).copy()
    wo_i = w_out[0, HALF:].reshape(HALF, 1).copy()
    ident = np.eye(128, dtype=np.float32).astype(ml_dtypes.bfloat16)

    tpu = _tpos_u(src)
    tpi = _tpos_i(dst)

    in_maps, metas = [], []
    for k in range(N_CORES):
        zu_sh = z_user[k * U_OWN:min((k + 1) * U_OWN, N_USERS)]
        zi_sh = z_item[k * I_OWN:min((k + 1) * I_OWN, N_ITEMS)]
        m = _pack_core2(tpu[k * EC:(k + 1) * EC], tpi[k * EC:(k + 1) * EC])
        metas.append(m)
        in_maps.append({
            "zt": _swizzle_zt(zu_sh, zi_sh),
            "w_user": w_user, "w_item": w_item,
            "wo_u": wo_u, "wo_i": wo_i,
            "b_user": b_user, "b_item": b_item, "b_out": b_out,
            "identd": ident,
            "lsa": m["lsa"], "lsb": m["lsb"],
            "p1i": m["p1i"], "p2i": m["p2i"],
            "ma": m["ma"].astype(ml_dtypes.bfloat16),
            "mb": m["mb"].astype(ml_dtypes.bfloat16),
        })
    return in_maps, metas


def _run(inputs, trace=False):
    from concourse.bass_utils import run_bass_kernel_spmd

    nc = _build()
    in_maps, metas = _make_in_maps(inputs)
    res = run_bass_kernel_spmd(
        nc, in_maps, core_ids=list(range(N_CORES)), trace=trace
    )
    full = np.empty(E, dtype=np.float32)
    for k in range(N_CORES):
        o = res.results[k]["out"]
        m = metas[k]
        full[k * EC:(k + 1) * EC] = o[m["paE"], m["jaE"]]
    return full.reshape(E, 1), res


def kernel(**inputs):
    full, _ = _run(inputs, trace=False)
    return full


# revision 35
# speedup vs baseline: 1.2036x; 1.2036x over previous
"""Trainium2 Bass kernel for ContextAwareArtRecSys (gnn_message_passing).

Math fold: the reference is
    score[e] = concat(z_u[src] @ Wu.T + bu, z_i[dst] @ Wi.T + bi) @ wo.T + bo
Everything after the gather is linear, so with
    vu = wo[:, :128] @ Wu,  vi = wo[:, 128:] @ Wi          (256-vectors)
    c  = wo[:, :128]@bu + wo[:, 128:]@bi + bo              (scalar)
we have score[e] = (z_u @ vu)[src] + (z_i @ vi)[dst] + c.

v3 design — no per-edge indirect-DMA descriptors (SWDGE service is
hard-serialized at ~4.2ns/descriptor, measured):

  1. Node scores z@v on the PE: z is shipped transposed in 512-node
     chunks, v is a 1-column stationary, each chunk accumulates h-halves
     into a [1,512] PSUM row which the Activation engine drains to a
     bf16 score row in SBUF.  One combined AllGather publishes every
     core's (users | items) score block.
  2. Each core covers 62,500 edges in TWO row-bucketed layouts:
     space A rows = user-table position // 392 (user side), space B
     rows = item-table position // 784.  Each row's node scores are a
     STATIC contiguous slice of the AllGathered table -> plain DMAs.
  3. Per-row expansion: local_scatter (GPSIMD ucode, per-partition int16
     indices, ~4.3ns/idx) drops each node score at its first-edge slot,
     then a masked DVE scan (state = M*state + V) fills each segment.
  4. The item side is routed from space B to space A with
     local_scatter -> 16 PE tile-transposes -> local_scatter
     (any (q,j)->(p,f) permutation = row-permute, transpose, row-permute).
  5. One fused DVE op adds user + item + c; result DMAs out.
"""

import numpy as np

N_CORES = 8
N_USERS, N_ITEMS, E, H = 50000, 100000, 500000, 256
HALF = H // 2
EC = E // N_CORES                  # 62500 edges per core

U_OWN = 6272                       # users owned per core (8*6272 = 50176)
I_OWN = 12544                      # items owned per core (8*12544 = 100352)
S_OWN = U_OWN + I_OWN              # 18816 per-core score block
UCH = 13                           # user psum chunks of 512 (6656 slots)
ICH = 25                           # item psum chunks of 512 (12800 slots)
TCH = UCH + ICH                    # 38
USLOT = ICH * 512                  # 12800: user slots start (items first)
UA = 392                           # user-table entries per A-row (16/core)
IA = 784                           # item-table entries per B-row (16/core)
F = 640                            # edge slots per row
STG = 2046                         # staging cols (local_scatter dst < 2048)
STG2 = 2048                        # post-transpose staging cols

_CACHE = {}


def _build():
    if "nc" in _CACHE:
        return _CACHE["nc"]
    import concourse.bass as bass
    import concourse.tile as tile
    import concourse.mybir as mybir
    from concourse import bacc

    f32 = mybir.dt.float32
    bf16 = mybir.dt.bfloat16
    i16 = mybir.dt.int16

    nc = bacc.Bacc("TRN2", target_bir_lowering=False, debug=False,
                   num_devices=N_CORES)

    zt = nc.dram_tensor("zt", [128, TCH * 1024], bf16, kind="ExternalInput")
    w_user = nc.dram_tensor("w_user", [HALF, H], f32, kind="ExternalInput")
    w_item = nc.dram_tensor("w_item", [HALF, H], f32, kind="ExternalInput")
    wo_u = nc.dram_tensor("wo_u", [HALF, 1], f32, kind="ExternalInput")
    wo_i = nc.dram_tensor("wo_i", [HALF, 1], f32, kind="ExternalInput")
    b_user = nc.dram_tensor("b_user", [HALF, 1], f32, kind="ExternalInput")
    b_item = nc.dram_tensor("b_item", [HALF, 1], f32, kind="ExternalInput")
    b_out = nc.dram_tensor("b_out", [1, 1], f32, kind="ExternalInput")
    identd = nc.dram_tensor("identd", [128, 128], bf16, kind="ExternalInput")
    lsa = nc.dram_tensor("lsa", [128, UA], i16, kind="ExternalInput")
    lsb = nc.dram_tensor("lsb", [128, IA], i16, kind="ExternalInput")
    p1i = nc.dram_tensor("p1i", [128, F], i16, kind="ExternalInput")
    p2i = nc.dram_tensor("p2i", [128, STG2], i16, kind="ExternalInput")
    ma = nc.dram_tensor("ma", [128, F], bf16, kind="ExternalInput")
    mb = nc.dram_tensor("mb", [128, F], bf16, kind="ExternalInput")
    out = nc.dram_tensor("out", [128, F], f32, kind="ExternalOutput")

    warm_loc = nc.dram_tensor("warm_loc", [N_CORES, 1], bf16)
    warm_all = nc.dram_tensor("warm_all", [N_CORES, 1], bf16)
    s_loc = nc.dram_tensor("s_loc", [N_CORES * S_OWN, 1], bf16)
    s_all = nc.dram_tensor("s_all", [N_CORES * S_OWN, 1], bf16)

    groups = [list(range(N_CORES))]

    with tile.TileContext(nc) as tc:
        with (
            tc.tile_pool(name="consts", bufs=1) as consts,
            tc.tile_pool(name="zpool", bufs=4) as zpool,
            tc.tile_pool(name="work", bufs=1) as work,
            tc.tile_pool(name="psum", bufs=1, space="PSUM") as psum,
            tc.tile_pool(name="spsum", bufs=4, space="PSUM") as spsum,
        ):
            # ---- warm up the collective engine (absorbs cross-core
            # sync + CC setup under phase 1) ----
            nc.gpsimd.collective_compute(
                "AllToAll", mybir.AluOpType.bypass,
                replica_groups=groups, ins=[warm_loc.ap()],
                outs=[warm_all.ap()],
            )

            # ---- weights for the fold ----
            wu_t = consts.tile([HALF, H], f32)
            nc.sync.dma_start(wu_t[:], w_user.ap())
            wi_t = consts.tile([HALF, H], f32)
            nc.sync.dma_start(wi_t[:], w_item.ap())
            wou_t = consts.tile([HALF, 1], f32)
            nc.sync.dma_start(wou_t[:], wo_u.ap())
            woi_t = consts.tile([HALF, 1], f32)
            nc.sync.dma_start(woi_t[:], wo_i.ap())
            bu_t = consts.tile([HALF, 1], f32)
            nc.sync.dma_start(bu_t[:], b_user.ap())
            bi_t = consts.tile([HALF, 1], f32)
            nc.sync.dma_start(bi_t[:], b_item.ap())
            bo_t = consts.tile([1, 1], f32)
            nc.sync.dma_start(bo_t[:], b_out.ap())


            # ---- fold: vT columns (vu0 | vu1 | vi0 | vi1), and c ----
            vps = psum.tile([128, 4], f32, tag="vps")
            nc.tensor.matmul(vps[:, 0:1], wu_t[:, 0:HALF], wou_t[:],
                             start=True, stop=True)
            nc.tensor.matmul(vps[:, 1:2], wu_t[:, HALF:H], wou_t[:],
                             start=True, stop=True)
            nc.tensor.matmul(vps[:, 2:3], wi_t[:, 0:HALF], woi_t[:],
                             start=True, stop=True)
            nc.tensor.matmul(vps[:, 3:4], wi_t[:, HALF:H], woi_t[:],
                             start=True, stop=True)
            vT = consts.tile([128, 4], bf16)
            nc.vector.tensor_copy(vT[:], vps[:])

            ones_k1 = consts.tile([HALF, 128], f32)
            nc.vector.memset(ones_k1[:], 1.0)
            bub = consts.tile([HALF, 128], f32)
            nc.vector.tensor_scalar_mul(bub[:], ones_k1[:], bu_t[:])
            bib = consts.tile([HALF, 128], f32)
            nc.vector.tensor_scalar_mul(bib[:], ones_k1[:], bi_t[:])
            cps = psum.tile([128, 1], f32, tag="cps")
            nc.tensor.matmul(cps[:], bub[:], wou_t[:], start=True, stop=False)
            nc.tensor.matmul(cps[:], bib[:], woi_t[:], start=False, stop=False)
            nc.tensor.matmul(cps[:], ones_k1[0:1, :], bo_t[:],
                             start=False, stop=True)
            c_t = consts.tile([128, 1], f32)
            nc.vector.tensor_copy(c_t[:], cps[:])

            # ---- node scores on PE, drained by Activation to bf16 row ----
            # chunks grouped 4 per DMA, alternating between the two HWDGE
            # queues (SP / Activation) to split the z bandwidth.
            srow = work.tile([1, TCH * 512], bf16)
            GRP = 4
            usl = work.tile([128, UA], bf16)
            ua_t = work.tile([128, F], bf16)
            uexp = work.tile([128, F], bf16)
            for g in range(0, TCH, GRP):
                n = min(GRP, TCH - g)
                zch = zpool.tile([128, GRP * 1024], bf16, tag="z")
                zeng = nc.sync if (g // GRP) % 2 == 0 else nc.scalar
                zeng.dma_start(
                    zch[:, 0:n * 1024],
                    zt.ap()[:, g * 1024:(g + n) * 1024],
                )
                pss = [spsum.tile([1, 512], f32, tag="sc", name=f"ps_{g}_{j}")
                       for j in range(n)]
                for j in range(n):
                    a = 2 if g + j < ICH else 0
                    nc.tensor.matmul(pss[j][:], vT[:, a:a + 1],
                                     zch[:, j * 1024:j * 1024 + 512],
                                     start=True, stop=False)
                for j in range(n):
                    a = 2 if g + j < ICH else 0
                    nc.tensor.matmul(pss[j][:], vT[:, a + 1:a + 2],
                                     zch[:, j * 1024 + 512:(j + 1) * 1024],
                                     start=False, stop=True)
                for j in range(n):
                    c = g + j
                    if c % 2 == 0:
                        nc.vector.tensor_copy(
                            srow[0:1, c * 512:(c + 1) * 512], pss[j][:])
                    else:
                        nc.scalar.copy(
                            srow[0:1, c * 512:(c + 1) * 512], pss[j][:])

            # one combined collective: per-replica block = [items | users].
            # Item halves on sync (drain earlier), user halves on scalar.
            for rr in range(N_CORES):
                nc.sync.dma_start(
                    s_loc.ap()[rr * S_OWN:rr * S_OWN + I_OWN]
                    .rearrange("(a b) one -> a (b one)", a=1),
                    srow[0:1, 0:I_OWN],
                )
            for rr in range(N_CORES):
                nc.scalar.dma_start(
                    s_loc.ap()[rr * S_OWN + I_OWN:(rr + 1) * S_OWN]
                    .rearrange("(a b) one -> a (b one)", a=1),
                    srow[0:1, USLOT:USLOT + U_OWN],
                )
            nc.gpsimd.collective_compute(
                "AllToAll", mybir.AluOpType.bypass,
                replica_groups=groups,
                ins=[s_loc.ap()],
                outs=[s_all.ap()],
            )

            # aux inputs for the expansion/routing phases (sync queue,
            # overlapping the collectives)
            ident = consts.tile([128, 128], bf16)
            nc.sync.dma_start(ident[:], identd.ap())
            lsa_t = consts.tile([128, UA], i16)
            nc.sync.dma_start(lsa_t[:], lsa.ap())
            lsb_t = consts.tile([128, IA], i16)
            nc.sync.dma_start(lsb_t[:], lsb.ap())
            p1_t = consts.tile([128, F], i16)
            nc.sync.dma_start(p1_t[:], p1i.ap())
            p2_t = consts.tile([128, STG2], i16)
            nc.sync.dma_start(p2_t[:], p2i.ap())
            ma_t = consts.tile([128, F], bf16)
            nc.sync.dma_start(ma_t[:], ma.ap())
            mb_t = consts.tile([128, F], bf16)
            nc.sync.dma_start(mb_t[:], mb.ap())
            sall2 = s_all.ap().rearrange("(k c) one -> k (c one)", k=N_CORES)
            isl = work.tile([128, IA], bf16)
            nc.sync.dma_start(
                isl[:],
                sall2[:, 0:I_OWN].rearrange("k (r a) -> k r a", r=16),
            )
            nc.scalar.dma_start(
                usl[:],
                sall2[:, I_OWN:S_OWN].rearrange("k (r a) -> k r a", r=16),
            )

            # ---- item side: seg-start scatter + scan (space B) ----
            ib_t = work.tile([128, F], bf16)
            nc.gpsimd.local_scatter(ib_t[:], isl[:], lsb_t[:], 128, F, IA)
            iexp = work.tile([128, F], bf16)
            nc.vector.tensor_tensor_scan(
                iexp[:], mb_t[:], ib_t[:], 0.0,
                mybir.AluOpType.mult, mybir.AluOpType.add,
            )

            # ---- route item scores B -> A: ls, transpose, ls ----
            stg = work.tile([128, STG], bf16)
            nc.gpsimd.local_scatter(stg[:], iexp[:], p1_t[:], 128, STG, F)
            # user expansion fills the GPSIMD gap while PE transposes run
            nc.gpsimd.local_scatter(ua_t[:], usl[:], lsa_t[:], 128, F, UA)
            nc.vector.tensor_tensor_scan(
                uexp[:], ma_t[:], ua_t[:], 0.0,
                mybir.AluOpType.mult, mybir.AluOpType.add,
            )
            pt = psum.tile([128, STG2], bf16, tag="pt")
            for t in range(15):
                nc.tensor.transpose(
                    pt[:, t * 128:(t + 1) * 128],
                    stg[:, t * 128:(t + 1) * 128], ident[:]
                )
            nc.tensor.transpose(
                pt[0:STG - 1920, 1920:2048], stg[:, 1920:STG], ident[:]
            )
            stg2 = work.tile([128, STG2], bf16)
            nc.vector.tensor_copy(stg2[:], pt[:])
            iex2 = work.tile([128, F], bf16)
            nc.gpsimd.local_scatter(iex2[:], stg2[:], p2_t[:], 128, F, STG2)

            # ---- combine: out = (uexp + c) + iex2 ----
            outf = work.tile([128, F], f32)
            nc.vector.scalar_tensor_tensor(
                outf[:], uexp[:], c_t[:], iex2[:],
                mybir.AluOpType.add, mybir.AluOpType.add,
            )
            nc.sync.dma_start(out.ap(), outf[:])

    nc.compile()
    _CACHE["nc"] = nc
    return nc


def _swizzle_zt(zu_sh, zi_sh):
    """[128, TCH*1024] bf16: chunk c = 512-node block; cols = h0|h1 halves.
    zT[h, 1024c + half*512 + j] = z[node(512c + j), half*128 + h]."""
    import ml_dtypes

    zu = np.zeros((UCH * 512, H), dtype=np.float32)
    zu[:zu_sh.shape[0]] = zu_sh
    zi = np.zeros((ICH * 512, H), dtype=np.float32)
    zi[:zi_sh.shape[0]] = zi_sh
    zall = np.concatenate([zi, zu], axis=0)            # items first
    zr = zall.reshape(TCH, 512, 2, 128).transpose(3, 0, 2, 1)
    return np.ascontiguousarray(
        zr.reshape(128, TCH * 1024)
    ).astype(ml_dtypes.bfloat16)


def _tpos_u(u):
    return S_OWN * (u // U_OWN) + I_OWN + u % U_OWN


def _tpos_i(d):
    return S_OWN * (d // I_OWN) + d % I_OWN


def _rank_in_group(keys):
    order = np.argsort(keys, kind="stable")
    ks = keys[order]
    first = np.r_[True, ks[1:] != ks[:-1]]
    gstart = np.where(first)[0]
    ranks_sorted = np.arange(len(keys)) - np.repeat(
        gstart, np.diff(np.r_[gstart, len(keys)])
    )
    ranks = np.empty(len(keys), dtype=np.int64)
    ranks[order] = ranks_sorted
    return ranks


def _pack_core2(tpu, tpi):
    """Host-side index construction for one core's edges, from the
    table positions (tpu/tpi) of each edge's endpoints."""
    # user side rows: tpos = S_OWN*k + local, local < U_OWN
    la = tpu % S_OWN - I_OWN
    paE = 16 * (tpu // S_OWN) + la // UA
    oaE = la % UA                                # offset within row slice
    keyA = paE * (UA + 1) + oaE                  # sort key: (row, offset)
    orderA = np.argsort(keyA, kind="stable")
    paS = paE[orderA]
    oaS = oaE[orderA]
    rowstartA = np.searchsorted(paS, np.arange(128))
    jaS = np.arange(EC) - rowstartA[paS]
    assert jaS.max() < F, f"A row overflow {jaS.max()}"
    firstA = np.r_[True, (paS[1:] != paS[:-1]) | (oaS[1:] != oaS[:-1])]
    lsa_m = np.full((128, UA), -1, dtype=np.int16)
    lsa_m[paS[firstA], oaS[firstA]] = jaS[firstA]
    ma_m = np.ones((128, F), dtype=np.float32)
    ma_m[paS[firstA], jaS[firstA]] = 0.0

    posA = np.empty(EC, dtype=np.int64)
    posA[orderA] = np.arange(EC)
    jaE = posA - rowstartA[paE]

    # item side rows
    lb = tpi % S_OWN
    qbE = 16 * (tpi // S_OWN) + lb // IA
    obE = lb % IA
    keyB = qbE * (IA + 1) + obE
    orderB = np.argsort(keyB, kind="stable")
    qbS = qbE[orderB]
    obS = obE[orderB]
    rowstartB = np.searchsorted(qbS, np.arange(128))
    jbS = np.arange(EC) - rowstartB[qbS]
    assert jbS.max() < F, f"B row overflow {jbS.max()}"
    firstB = np.r_[True, (qbS[1:] != qbS[:-1]) | (obS[1:] != obS[:-1])]
    lsb_m = np.full((128, IA), -1, dtype=np.int16)
    lsb_m[qbS[firstB], obS[firstB]] = jbS[firstB]
    mb_m = np.ones((128, F), dtype=np.float32)
    mb_m[qbS[firstB], jbS[firstB]] = 0.0

    posB = np.empty(EC, dtype=np.int64)
    posB[orderB] = np.arange(EC)
    jbE = posB - rowstartB[qbE]

    # routing: staging col = 128*t + paE on row qbE; after transpose the
    # value sits at (paE, 128*t + qbE) and moves to final slot jaE.
    tE = _rank_in_group(qbE * 128 + paE)
    scol = 128 * tE + paE
    assert scol.max() < STG, f"staging overflow {scol.max()}"
    p1_m = np.full((128, F), -1, dtype=np.int16)
    p1_m[qbE, jbE] = scol
    p2_m = np.full((128, STG2), -1, dtype=np.int16)
    p2_m[paE, 128 * tE + qbE] = jaE

    return {
        "lsa": lsa_m, "lsb": lsb_m, "p1i": p1_m, "p2i": p2_m,
        "ma": ma_m, "mb": mb_m, "paE": paE, "jaE": jaE,
    }


def _make_in_maps(inputs):
    import ml_dtypes

    z_user = np.asarray(inputs["z_user"], dtype=np.float32)
    z_item = np.asarray(inputs["z_item"], dtype=np.float32)
    src = np.asarray(inputs["edge_src"]).astype(np.int64)
    dst = np.asarray(inputs["edge_dst"]).astype(np.int64)
    w_user = np.asarray(inputs["w_user"], dtype=np.float32)
    w_item = np.asarray(inputs["w_item"], dtype=np.float32)
    b_user = np.asarray(inputs["b_user"], dtype=np.float32).reshape(HALF, 1)
    b_item = np.asarray(inputs["b_item"], dtype=np.float32).reshape(HALF, 1)
    w_out = np.asarray(inputs["w_out"], dtype=np.float32)
    b_out = np.asarray(inputs["b_out"], dtype=np.float32).reshape(1, 1)
    wo_u = w_out[0, :HALF].reshape(HALF, 1).copy()
    wo_i = w_out[0, HALF:].reshape(HALF, 1).copy()
    ident = np.eye(128, dtype=np.float32).astype(ml_dtypes.bfloat16)

    tpu = _tpos_u(src)
    tpi = _tpos_i(dst)

    in_maps, metas = [], []
    for k in range(N_CORES):
        zu_sh = z_user[k * U_OWN:min((k + 1) * U_OWN, N_USERS)]
        zi_sh = z_item[k * I_OWN:min((k + 1) * I_OWN, N_ITEMS)]
        m = _pack_core2(tpu[k * EC:(k + 1) * EC], tpi[k * EC:(k + 1) * EC])
        metas.append(m)
        in_maps.append({
            "zt": _swizzle_zt(zu_sh, zi_sh),
            "w_user": w_user, "w_item": w_item,
            "wo_u": wo_u, "wo_i": wo_i,
            "b_user": b_user, "b_item": b_item, "b_out": b_out,
            "identd": ident,
            "lsa": m["lsa"], "lsb": m["lsb"],
            "p1i": m["p1i"], "p2i": m["p2i"],
            "ma": m["ma"].astype(ml_dtypes.bfloat16),
            "mb": m["mb"].astype(ml_dtypes.bfloat16),
        })
    return in_maps, metas


def _run(inputs, trace=False):
    from concourse.bass_utils import run_bass_kernel_spmd

    nc = _build()
    in_maps, metas = _make_in_maps(inputs)
    res = run_bass_kernel_spmd(
        nc, in_maps, core_ids=list(range(N_CORES)), trace=trace
    )
    full = np.empty(E, dtype=np.float32)
    for k in range(N_CORES):
        o = res.results[k]["out"]
        m = metas[k]
        full[k * EC:(k + 1) * EC] = o[m["paE"], m["jaE"]]
    return full.reshape(E, 1), res


def kernel(**inputs):
    full, _ = _run(inputs, trace=False)
    return full
